# revision 6
# baseline (speedup 1.0000x reference)
"""CSI decoder kernel: LayerNorm(d) -> Linear(512->2) -> per-subcarrier scale -> complex.

Distribution: pure data parallel over 8 NeuronCores. The flattened token axis
(B*A_bs*A_ue*S = 262144 tokens) is split contiguously into 8 shards of 32768
tokens; each core reads its 64 MiB x-shard and produces [32768, 2] f32
(real, imag interleaved), gathered host-side into the complex64 output.

Math: the normalized tensor is never materialized. With
  Wg[o] = W[o] * gamma,  sW[o] = sum(Wg[o]),  c[o] = W[o] @ beta + b[o]
each token needs only 4 reductions over d:
  s1 = sum(x), s2 = sum(x^2), p_o = x . Wg[o]
  mu = s1/512, var = s2/512 - mu^2, rstd = 1/sqrt(var+eps)
  cf_o = rstd*(p_o - mu*sW[o]) + c[o];  out = cf * (|scalers_s| + 0.1)

On-chip layout: within a core's 32768-token shard, token t maps to
(partition p, column j) = (t // 256, t % 256), so both the input and output
DMAs see large contiguous per-partition runs. Per 128x512 token-tile:
ScalarE does one Copy-with-accum pass (bf16 cast + s1), VectorE does the two
dot products via fused tensor_tensor_reduce on bf16; sum(x^2) is split
between ScalarE (Square+accum on f32) and VectorE (TTR x*x on bf16) to
balance the engines. Epilogue runs once on [128, 256] stat buffers.
"""

from contextlib import ExitStack

import ml_dtypes
import numpy as np

import concourse.bass as bass
import concourse.tile as tile
from concourse import mybir
from concourse.bass_utils import run_bass_kernel_spmd

N_CORES = 8
B, A_BS, A_UE, S, D = 16, 64, 4, 64, 512
TOKENS = B * A_BS * A_UE * S            # 262144
TOK_PER_CORE = TOKENS // N_CORES        # 32768
NTILES = TOK_PER_CORE // 128            # 256 token-tiles of [128, 512]
CHUNK = 8                               # token-tiles per input DMA (2 MiB)
NCHUNKS = NTILES // CHUNK               # 32
EPS = 1e-5
# within each chunk, tiles [0, DVE_S2_TILES) compute sum(x^2) on VectorE,
# the rest on ScalarE (engine balance knob)
DVE_S2_TILES = 5

F32 = mybir.dt.float32
BF16 = mybir.dt.bfloat16
ALU = mybir.AluOpType
AF = mybir.ActivationFunctionType


def _split_multi_waits(nc):
    """Workaround for this walrus build: an instruction may carry at most one
    embedded sync wait; hoist extras into standalone no-ops placed before it."""
    for bbobj in nc.bb_map.values():
        insts = list(bbobj.bb.instructions)
        changed = False
        new_list = []
        for inst in insts:
            si = inst.sync_info
            if si is not None and si.on_wait is not None and len(si.on_wait) > 1:
                waits = list(si.on_wait)
                si.on_wait = waits[:1]
                for i, w in enumerate(waits[1:]):
                    nop = mybir.InstNoOp(name=f"wsplit_{inst.name}_{i}")
                    nop.engine = inst.engine
                    nop.sync_info = mybir.SyncInfo(on_wait=[w], on_update=[])
                    try:
                        nc.register_instruction(nop, overwrite=True)
                    except Exception:
                        pass
                    new_list.append(nop)
                changed = True
            new_list.append(inst)
        if changed:
            bbobj.bb.instructions = new_list


def _build(sw0: float, sw1: float, c0: float, c1: float):
    nc = bass.Bass(
        "TRN2", target_bir_lowering=False, debug=False, num_devices=N_CORES
    )
    x_in = nc.dram_tensor("x", [TOK_PER_CORE, D], F32, kind="ExternalInput")
    wg_in = nc.dram_tensor("wg", [128, 2 * D], BF16, kind="ExternalInput")
    sc_in = nc.dram_tensor("sc", [128, NTILES], F32, kind="ExternalInput")
    out_t = nc.dram_tensor("out", [TOK_PER_CORE, 2], F32, kind="ExternalOutput")

    # token t = p*NTILES + j  ->  [partition, tile-column, d]
    x_v = x_in.ap().rearrange("(p j) d -> p j d", p=128)
    out_v = out_t.ap().rearrange("(p j) two -> p j two", p=128)

    with tile.TileContext(nc) as tc, ExitStack() as ctx:
        const_pool = ctx.enter_context(tc.tile_pool(name="const", bufs=1))
        xf_pool = ctx.enter_context(tc.tile_pool(name="xf", bufs=3))
        xb_pool = ctx.enter_context(tc.tile_pool(name="xb", bufs=3 * CHUNK))
        stat_pool = ctx.enter_context(tc.tile_pool(name="stat", bufs=1))
        ep_pool = ctx.enter_context(tc.tile_pool(name="ep", bufs=1))

        wg = const_pool.tile([128, 2 * D], BF16)
        nc.sync.dma_start(out=wg[:], in_=wg_in.ap())
        sc = const_pool.tile([128, NTILES], F32)
        nc.sync.dma_start(out=sc[:], in_=sc_in.ap())

        s1B = stat_pool.tile([128, NTILES], F32)
        p0B = stat_pool.tile([128, NTILES], F32)
        p1B = stat_pool.tile([128, NTILES], F32)
        s2B = stat_pool.tile([128, NTILES], F32)
        act_junk = stat_pool.tile([128, D], BF16)
        dve_junk = stat_pool.tile([128, 1], BF16)

        for ci in range(NCHUNKS):
            xf = xf_pool.tile([128, CHUNK, D], F32)
            nc.sync.dma_start(
                out=xf[:], in_=x_v[:, ci * CHUNK : (ci + 1) * CHUNK, :]
            )
            for t in range(CHUNK):
                j = ci * CHUNK + t
                xt = xf[:, t, :]
                xb = xb_pool.tile([128, D], BF16)
                # cast to bf16 + s1 accumulation in one ScalarE pass
                nc.scalar.activation(
                    out=xb[:], in_=xt, func=AF.Copy,
                    accum_out=s1B[:, j : j + 1],
                )
                nc.vector.scalar_tensor_tensor(
                    out=dve_junk.broadcast_to([128, D]), in0=xb[:], scalar=1.0,
                    in1=wg[:, 0:D], op0=ALU.mult, op1=ALU.mult,
                    accum_out=p0B[:, j : j + 1],
                )
                nc.vector.scalar_tensor_tensor(
                    out=dve_junk.broadcast_to([128, D]), in0=xb[:], scalar=1.0,
                    in1=wg[:, D : 2 * D], op0=ALU.mult, op1=ALU.mult,
                    accum_out=p1B[:, j : j + 1],
                )
                if t < DVE_S2_TILES:
                    nc.vector.scalar_tensor_tensor(
                        out=dve_junk.broadcast_to([128, D]), in0=xb[:], scalar=1.0,
                        in1=xb[:], op0=ALU.mult, op1=ALU.mult,
                        accum_out=s2B[:, j : j + 1],
                    )
                else:
                    nc.scalar.activation(
                        out=act_junk[:], in_=xt, func=AF.Square,
                        accum_out=s2B[:, j : j + 1],
                    )

        # ---- epilogue on [128, NTILES] stat buffers ----
        mu = ep_pool.tile([128, NTILES], F32)
        nc.vector.tensor_scalar_mul(mu[:], s1B[:], 1.0 / D)
        # ex2e = s2/512 + eps
        ex2 = ep_pool.tile([128, NTILES], F32)
        nc.vector.tensor_scalar(
            out=ex2[:], in0=s2B[:], scalar1=1.0 / D, scalar2=EPS,
            op0=ALU.mult, op1=ALU.add,
        )
        nmu = ep_pool.tile([128, NTILES], F32)
        nc.vector.tensor_scalar_mul(nmu[:], mu[:], -1.0)
        var = ep_pool.tile([128, NTILES], F32)
        nc.vector.scalar_tensor_tensor(
            out=var[:], in0=mu[:], scalar=1.0, in1=nmu[:],
            op0=ALU.mult, op1=ALU.mult,
        )
        nc.vector.tensor_tensor(out=var[:], in0=var[:], in1=ex2[:], op=ALU.add)
        # rstd = 1/sqrt(var+eps)
        sd = ep_pool.tile([128, NTILES], F32)
        nc.scalar.activation(out=sd[:], in_=var[:], func=AF.Sqrt)
        rstd = ep_pool.tile([128, NTILES], F32)
        nc.vector.reciprocal(rstd[:], sd[:])

        outB = ep_pool.tile([128, NTILES, 2], F32)
        for o, (pB, sw, c) in enumerate(((p0B, sw0, c0), (p1B, sw1, c1))):
            a = ep_pool.tile([128, NTILES], F32, tag="ep_a")
            # a = p - mu*sW
            nc.vector.scalar_tensor_tensor(
                out=a[:], in0=mu[:], scalar=-sw, in1=pB[:],
                op0=ALU.mult, op1=ALU.add,
            )
            cf = ep_pool.tile([128, NTILES], F32, tag="ep_cf")
            nc.vector.tensor_tensor(out=cf[:], in0=a[:], in1=rstd[:], op=ALU.mult)
            # out = (cf + c) * scale   (scale varies along the free axis)
            nc.vector.scalar_tensor_tensor(
                out=outB[:, :, o], in0=cf[:], scalar=c, in1=sc[:],
                op0=ALU.add, op1=ALU.mult,
            )

        nc.sync.dma_start(out=out_v, in_=outB[:])
    _split_multi_waits(nc)
    return nc


def _prepare(x, ln_gamma, ln_beta, W, b, scalers):
    x = np.asarray(x, dtype=np.float32)
    ln_gamma = np.asarray(ln_gamma, dtype=np.float32)
    ln_beta = np.asarray(ln_beta, dtype=np.float32)
    W = np.asarray(W, dtype=np.float32)
    b = np.asarray(b, dtype=np.float32)
    scalers = np.asarray(scalers, dtype=np.float32)

    wg = W * ln_gamma[None, :]                      # [2, 512]
    sw = wg.sum(axis=1)                             # [2]
    c = W @ ln_beta + b                             # [2]
    wg_rep = np.ascontiguousarray(
        np.broadcast_to(
            np.concatenate([wg[0], wg[1]])[None, :], (128, 2 * D)
        ).astype(ml_dtypes.bfloat16)
    )
    # token t = p*NTILES + j ; subcarrier s = t % 64 = j % 64 (NTILES % 64 == 0)
    scale = np.abs(scalers) + 0.1                   # [64]
    sc_row = scale[(np.arange(NTILES) % S)].astype(np.float32)
    sc_rep = np.ascontiguousarray(
        np.broadcast_to(sc_row[None, :], (128, NTILES))
    )

    nc = _build(float(sw[0]), float(sw[1]), float(c[0]), float(c[1]))

    x_flat = np.ascontiguousarray(x.reshape(TOKENS, D))
    in_maps = [
        {
            "x": x_flat[i * TOK_PER_CORE : (i + 1) * TOK_PER_CORE],
            "wg": wg_rep,
            "sc": sc_rep,
        }
        for i in range(N_CORES)
    ]
    return nc, in_maps


def kernel(x, ln_gamma, ln_beta, W, b, scalers):
    nc, in_maps = _prepare(x, ln_gamma, ln_beta, W, b, scalers)
    res = run_bass_kernel_spmd(nc, in_maps, core_ids=list(range(N_CORES)))
    out = np.concatenate([res.results[i]["out"] for i in range(N_CORES)], axis=0)
    out = np.ascontiguousarray(out.astype(np.float32))
    return out.view(np.complex64).reshape(B, A_BS, A_UE, S)


# revision 10
# speedup vs baseline: 2.5025x; 2.5025x over previous
"""CSI decoder kernel: LayerNorm(d) -> Linear(512->2) -> per-subcarrier scale -> complex.

Distribution: pure data parallel over 8 NeuronCores. The flattened token axis
(B*A_bs*A_ue*S = 262144 tokens) is split contiguously into 8 shards of 32768
tokens; each core reads its 64 MiB x-shard and produces [32768, 2] f32
(real, imag interleaved), gathered host-side into the complex64 output.

Math: the normalized tensor is never materialized. With
  Wg[o] = W[o] * gamma,  sW[o] = sum(Wg[o]),  c[o] = W[o] @ beta + b[o]
each token needs only 4 reductions over d:
  s1 = sum(x), s2 = sum(x^2), p_o = x . Wg[o]
  mu = s1/512, var = s2/512 - mu^2, rstd = 1/sqrt(var+eps)
  cf_o = rstd*(p_o - mu*sW[o]) + c[o];  out = cf * (|scalers_s| + 0.1)

v2 engine split (v1 was VectorE-bound at 88% busy):
  TensorE   transposes each [128 tok, 128 d] block to PSUM (f32), then a
            [d x tok]^T @ [d x 3] matmul against [Wg0|Wg1|ones] accumulates
            p0, p1, s1 into PSUM with tokens on partitions.
  ScalarE   copies transposed PSUM -> SBUF as bf16 (matmul stationary input).
  VectorE   sum(x^2) via fused scalar_tensor_tensor on the natural-layout f32
            tile, plus the batched epilogue.

On-chip layout: within a core's 32768-token shard, token t maps to
(partition p, column j) = (t // 256, t % 256), so the input and output DMAs
see large contiguous per-partition runs.
"""

from contextlib import ExitStack

import ml_dtypes
import numpy as np

import concourse.bass as bass
import concourse.tile as tile
from concourse import mybir
from concourse.bass_utils import run_bass_kernel_spmd

N_CORES = 8
B, A_BS, A_UE, S, D = 16, 64, 4, 64, 512
TOKENS = B * A_BS * A_UE * S            # 262144
TOK_PER_CORE = TOKENS // N_CORES        # 32768
NTILES = TOK_PER_CORE // 128            # 256 token-tiles of [128, 512]
CHUNK = 8                               # token-tiles per input DMA (2 MiB)
NCHUNKS = NTILES // CHUNK               # 32
EPS = 1e-5
NDCH = D // 128                         # 4 d-chunks per token-tile

F32 = mybir.dt.float32
BF16 = mybir.dt.bfloat16
ALU = mybir.AluOpType
AF = mybir.ActivationFunctionType


def _split_multi_waits(nc):
    """Workaround for this walrus build: an instruction may carry at most one
    embedded sync wait; hoist extras into standalone no-ops placed before it."""
    for bbobj in nc.bb_map.values():
        insts = list(bbobj.bb.instructions)
        changed = False
        new_list = []
        for inst in insts:
            si = inst.sync_info
            if si is not None and si.on_wait is not None and len(si.on_wait) > 1:
                waits = list(si.on_wait)
                si.on_wait = waits[:1]
                for i, w in enumerate(waits[1:]):
                    nop = mybir.InstNoOp(name=f"wsplit_{inst.name}_{i}")
                    nop.engine = inst.engine
                    nop.sync_info = mybir.SyncInfo(on_wait=[w], on_update=[])
                    try:
                        nc.register_instruction(nop, overwrite=True)
                    except Exception:
                        pass
                    new_list.append(nop)
                changed = True
            new_list.append(inst)
        if changed:
            bbobj.bb.instructions = new_list


def _build(sw0: float, sw1: float, c0: float, c1: float):
    nc = bass.Bass(
        "TRN2", target_bir_lowering=False, debug=False, num_devices=N_CORES
    )
    x_in = nc.dram_tensor("x", [TOK_PER_CORE, D], F32, kind="ExternalInput")
    ident_in = nc.dram_tensor("ident", [128, 128], F32, kind="ExternalInput")
    wst_in = nc.dram_tensor("wst", [128, NDCH, 4], BF16, kind="ExternalInput")
    sc_in = nc.dram_tensor("sc", [128, NTILES], F32, kind="ExternalInput")
    out_t = nc.dram_tensor("out", [TOK_PER_CORE, 2], F32, kind="ExternalOutput")

    # token t = p*NTILES + j  ->  [partition, tile-column, d]
    x_v = x_in.ap().rearrange("(p j) d -> p j d", p=128)
    out_v = out_t.ap().rearrange("(p j) two -> p j two", p=128)

    with tile.TileContext(nc) as tc, ExitStack() as ctx:
        const_pool = ctx.enter_context(tc.tile_pool(name="const", bufs=1))
        xf_pool = ctx.enter_context(tc.tile_pool(name="xf", bufs=3))
        xtb_pool = ctx.enter_context(tc.tile_pool(name="xtb", bufs=3 * CHUNK))
        ps_pool = ctx.enter_context(tc.tile_pool(name="ps", bufs=4, space="PSUM"))
        st_pool = ctx.enter_context(tc.tile_pool(name="st", bufs=1, space="PSUM"))
        stat_pool = ctx.enter_context(tc.tile_pool(name="stat", bufs=1))
        ep_pool = ctx.enter_context(tc.tile_pool(name="ep", bufs=1))

        ident = const_pool.tile([128, 128], F32)
        nc.sync.dma_start(out=ident[:], in_=ident_in.ap())
        wst = const_pool.tile([128, NDCH, 4], BF16)
        nc.sync.dma_start(out=wst[:], in_=wst_in.ap())
        sc = const_pool.tile([128, NTILES], F32)
        nc.sync.dma_start(out=sc[:], in_=sc_in.ap())

        # stats PSUM: one bank per 128 token-tiles; slot j%128 is 4 wide
        stats_ps = [
            st_pool.tile([128, 512], F32, name=f"stats_ps{h}", tag=f"stats_ps{h}")
            for h in range(NTILES // 128)
        ]

        s2B = stat_pool.tile([128, NTILES], F32)
        dve_junk = stat_pool.tile([128, 1], F32)

        for ci in range(NCHUNKS):
            xf = xf_pool.tile([128, CHUNK, D], F32)
            nc.sync.dma_start(
                out=xf[:], in_=x_v[:, ci * CHUNK : (ci + 1) * CHUNK, :]
            )
            for t in range(CHUNK):
                j = ci * CHUNK + t
                xt = xf[:, t, :]
                # transpose 4 [128 tok,128 d] blocks -> PSUM [128 d, 4, 128 tok]
                xtp = ps_pool.tile([128, NDCH, 128], F32)
                for c in range(NDCH):
                    nc.tensor.transpose(
                        out=xtp[:, c, :],
                        in_=xt[:, c * 128 : (c + 1) * 128],
                        identity=ident[:],
                    )
                # PSUM -> SBUF cast to bf16 (matmul stationary operand)
                xtb = xtb_pool.tile([128, NDCH, 128], BF16)
                nc.scalar.activation(out=xtb[:], in_=xtp[:], func=AF.Copy)
                # stats matmul: accumulate [tok, (p0,p1,s1)] over d-chunks
                slot = stats_ps[j // 128][:, (j % 128) * 4 : (j % 128) * 4 + 3]
                for c in range(NDCH):
                    nc.tensor.matmul(
                        out=slot,
                        lhsT=xtb[:, c, :],
                        rhs=wst[:, c, 0:3],
                        start=(c == 0),
                        stop=(c == NDCH - 1),
                    )
                # s2 on VectorE from the natural-layout f32 tile
                nc.vector.scalar_tensor_tensor(
                    out=dve_junk.broadcast_to([128, D]), in0=xt, scalar=1.0,
                    in1=xt, op0=ALU.mult, op1=ALU.mult,
                    accum_out=s2B[:, j : j + 1],
                )

        # ---- gather matmul stats PSUM -> SBUF ----
        stats_sb = ep_pool.tile([128, NTILES // 128, 512], F32)
        for h in range(NTILES // 128):
            nc.scalar.activation(
                out=stats_sb[:, h, :], in_=stats_ps[h][:], func=AF.Copy
            )
        p0B = ep_pool.tile([128, NTILES], F32)
        p1B = ep_pool.tile([128, NTILES], F32)
        s1B = ep_pool.tile([128, NTILES], F32)
        for h in range(NTILES // 128):
            quad = stats_sb[:, h, :].rearrange("p (j four) -> p j four", four=4)
            for k, dst in ((0, p0B), (1, p1B), (2, s1B)):
                nc.vector.tensor_copy(
                    out=dst[:, h * 128 : (h + 1) * 128], in_=quad[:, :, k]
                )

        # ---- epilogue on [128, NTILES] stat buffers ----
        mu = ep_pool.tile([128, NTILES], F32)
        nc.vector.tensor_scalar_mul(mu[:], s1B[:], 1.0 / D)
        # ex2e = s2/512 + eps
        ex2 = ep_pool.tile([128, NTILES], F32)
        nc.vector.tensor_scalar(
            out=ex2[:], in0=s2B[:], scalar1=1.0 / D, scalar2=EPS,
            op0=ALU.mult, op1=ALU.add,
        )
        nmu = ep_pool.tile([128, NTILES], F32)
        nc.vector.tensor_scalar_mul(nmu[:], mu[:], -1.0)
        var = ep_pool.tile([128, NTILES], F32)
        nc.vector.scalar_tensor_tensor(
            out=var[:], in0=mu[:], scalar=1.0, in1=nmu[:],
            op0=ALU.mult, op1=ALU.mult,
        )
        nc.vector.tensor_tensor(out=var[:], in0=var[:], in1=ex2[:], op=ALU.add)
        # rstd = 1/sqrt(var+eps)
        sd = ep_pool.tile([128, NTILES], F32)
        nc.scalar.activation(out=sd[:], in_=var[:], func=AF.Sqrt)
        rstd = ep_pool.tile([128, NTILES], F32)
        nc.vector.reciprocal(rstd[:], sd[:])

        outB = ep_pool.tile([128, NTILES, 2], F32)
        for o, (pB, sw, c) in enumerate(((p0B, sw0, c0), (p1B, sw1, c1))):
            a = ep_pool.tile([128, NTILES], F32, tag="ep_a")
            # a = p - mu*sW
            nc.vector.scalar_tensor_tensor(
                out=a[:], in0=mu[:], scalar=-sw, in1=pB[:],
                op0=ALU.mult, op1=ALU.add,
            )
            cf = ep_pool.tile([128, NTILES], F32, tag="ep_cf")
            nc.vector.tensor_tensor(out=cf[:], in0=a[:], in1=rstd[:], op=ALU.mult)
            # out = (cf + c) * scale   (scale varies along the free axis)
            nc.vector.scalar_tensor_tensor(
                out=outB[:, :, o], in0=cf[:], scalar=c, in1=sc[:],
                op0=ALU.add, op1=ALU.mult,
            )

        nc.sync.dma_start(out=out_v, in_=outB[:])
    _split_multi_waits(nc)
    return nc


def _prepare(x, ln_gamma, ln_beta, W, b, scalers):
    x = np.asarray(x, dtype=np.float32)
    ln_gamma = np.asarray(ln_gamma, dtype=np.float32)
    ln_beta = np.asarray(ln_beta, dtype=np.float32)
    W = np.asarray(W, dtype=np.float32)
    b = np.asarray(b, dtype=np.float32)
    scalers = np.asarray(scalers, dtype=np.float32)

    wg = W * ln_gamma[None, :]                      # [2, 512]
    sw = wg.sum(axis=1)                             # [2]
    c = W @ ln_beta + b                             # [2]
    # wst[k, c, :] = (Wg0[c*128+k], Wg1[c*128+k], 1, 0)
    wst = np.zeros((128, NDCH, 4), dtype=np.float32)
    wst[:, :, 0] = wg[0].reshape(NDCH, 128).T
    wst[:, :, 1] = wg[1].reshape(NDCH, 128).T
    wst[:, :, 2] = 1.0
    wst = np.ascontiguousarray(wst.astype(ml_dtypes.bfloat16))
    ident = np.ascontiguousarray(np.eye(128, dtype=np.float32))
    # token t = p*NTILES + j ; subcarrier s = t % 64 = j % 64 (NTILES % 64 == 0)
    scale = np.abs(scalers) + 0.1                   # [64]
    sc_row = scale[(np.arange(NTILES) % S)].astype(np.float32)
    sc_rep = np.ascontiguousarray(
        np.broadcast_to(sc_row[None, :], (128, NTILES))
    )

    nc = _build(float(sw[0]), float(sw[1]), float(c[0]), float(c[1]))

    x_flat = np.ascontiguousarray(x.reshape(TOKENS, D))
    in_maps = [
        {
            "x": x_flat[i * TOK_PER_CORE : (i + 1) * TOK_PER_CORE],
            "ident": ident,
            "wst": wst,
            "sc": sc_rep,
        }
        for i in range(N_CORES)
    ]
    return nc, in_maps


def kernel(x, ln_gamma, ln_beta, W, b, scalers):
    nc, in_maps = _prepare(x, ln_gamma, ln_beta, W, b, scalers)
    res = run_bass_kernel_spmd(nc, in_maps, core_ids=list(range(N_CORES)))
    out = np.concatenate([res.results[i]["out"] for i in range(N_CORES)], axis=0)
    out = np.ascontiguousarray(out.astype(np.float32))
    return out.view(np.complex64).reshape(B, A_BS, A_UE, S)
